# revision 3
# baseline (speedup 1.0000x reference)
"""Multi-head attention (QKV projection + softmax attention) on 8 TRN2 NeuronCores.

Reference computation (per full input):
    x: [2, 8, 4, 256, 768] fp32, H=12 heads, head_dim=64
    q = split_heads(x @ Wq.T + bq); k, v likewise
    out = softmax(q k^T / sqrt(64)) v, heads merged back -> [2, 8, 4, 256, 768]

Sharding: data-parallel over the 2*8*4 = 64 independent (b,t,l) sequences,
8 sequences per core; weights replicated.

Per-core kernel design (all matmuls bf16 inputs, fp32 PSUM accumulate):
  - x and W are cast fp32->bf16 during SWDGE DMA into DRAM scratch, then
    loaded transposed via the HWDGE xbar DMA-transpose, giving XT [c, tok]
    and WT [c, e] (contraction-major) in SBUF.
  - qT, kT computed feature-major ([e, tok]) so the q k^T matmul reads them
    directly; v computed token-major ([tok, e]) so attn @ v reads it directly.
  - v bias is added via a K=1 matmul row (ones lhsT, bv rhs) which is exact
    because softmax rows sum to one; q/k biases via per-partition
    tensor_scalar adds.
  - softmax skips the max-subtraction (logits are ~N(0,1); exp is safe) and
    gets its denominator for free from a ones-column appended to v, so
    normalization is one reciprocal + one per-partition scalar multiply.
"""

import sys

for _p in ("/opt/trn_rl_repo",):
    if _p not in sys.path:
        sys.path.insert(0, _p)

import numpy as np

import concourse.bass as bass
import concourse.tile as tile
from concourse import bacc, mybir
from concourse.bass_utils import run_bass_kernel_spmd

N_CORES = 8
B, T_, L, P_, D = 2, 8, 4, 256, 768
H = 12
HD = D // H          # 64
NSEQ = (B * T_ * L) // N_CORES   # 8 sequences per core
NT = NSEQ * P_       # 2048 tokens per core
CT = D // 128        # 6 contraction tiles
ET = D // 128        # 6 output-feature tiles
NCHUNK = 512         # moving-dim chunk for projections
SCALE = float(HD) ** -0.5

F32 = mybir.dt.float32
BF16 = mybir.dt.bfloat16


def build_nc():
    nc = bacc.Bacc()

    x = nc.dram_tensor("x", [NT, D], F32, kind="ExternalInput")
    Wq = nc.dram_tensor("Wq", [D, D], F32, kind="ExternalInput")
    Wk = nc.dram_tensor("Wk", [D, D], F32, kind="ExternalInput")
    Wv = nc.dram_tensor("Wv", [D, D], F32, kind="ExternalInput")
    bq = nc.dram_tensor("bq", [D], F32, kind="ExternalInput")
    bk = nc.dram_tensor("bk", [D], F32, kind="ExternalInput")
    bv = nc.dram_tensor("bv", [D], F32, kind="ExternalInput")
    out = nc.dram_tensor("out", [NT, D], F32, kind="ExternalOutput")

    # bf16 DRAM scratch for the xbar transpose loads
    xbf = nc.dram_tensor("xbf", [NT, D], BF16)
    wbf = {
        "q": nc.dram_tensor("wqbf", [D, D], BF16),
        "k": nc.dram_tensor("wkbf", [D, D], BF16),
        "v": nc.dram_tensor("wvbf", [D, D], BF16),
    }
    w_in = {"q": Wq, "k": Wk, "v": Wv}

    with tile.TileContext(nc) as tc:
        with (
            tc.tile_pool(name="const", bufs=1) as const,
            tc.tile_pool(name="big", bufs=1) as big,
            tc.tile_pool(name="attn", bufs=3) as attn_pool,
            tc.tile_pool(name="rec", bufs=8) as rec_pool,
            tc.tile_pool(name="outp", bufs=4) as out_pool,
            tc.tile_pool(name="ps_qk", bufs=2, space="PSUM") as ps_qk,
            tc.tile_pool(name="ps_v", bufs=2, space="PSUM") as ps_v,
            tc.tile_pool(name="ps_d", bufs=2, space="PSUM") as ps_d,
            tc.tile_pool(name="ps_av", bufs=2, space="PSUM") as ps_av,
        ):
            # ---- Phase 0: fp32 -> bf16 casts into DRAM scratch (SWDGE) ----
            nc.gpsimd.dma_start(out=xbf[:, :], in_=x[:, :])
            for wk in ("q", "k", "v"):
                nc.gpsimd.dma_start(out=wbf[wk][:, :], in_=w_in[wk][:, :])

            # biases: bq/bk as per-partition scalars [128, et]; bv as a bf16 row
            bqk = const.tile([128, 2, ET], F32)
            nc.sync.dma_start(out=bqk[:, 0, :], in_=bq.rearrange("(t p) -> p t", p=128))
            nc.sync.dma_start(out=bqk[:, 1, :], in_=bk.rearrange("(t p) -> p t", p=128))
            bvr = const.tile([1, D], BF16)
            nc.gpsimd.dma_start(out=bvr[:, :], in_=bv.rearrange("(o d) -> o d", o=1))
            ones = const.tile([1, 128], BF16)
            nc.vector.memset(ones[:, :], 1.0)

            # ---- Phase 1: transposed loads via xbar DMA-transpose ----
            XT = big.tile([128, CT, NT], BF16)     # XT[c%128, c//128, tok]
            for ct in range(CT):
                nc.sync.dma_start(
                    out=XT[:, ct, :],
                    in_=xbf[:, ct * 128:(ct + 1) * 128],
                    transpose=True,
                )
            WT = {}
            for wk in ("q", "k", "v"):
                WT[wk] = big.tile([128, CT, D], BF16, name=f"WT_{wk}")   # WT[c%128, c//128, e]
                for ct in range(CT):
                    nc.sync.dma_start(
                        out=WT[wk][:, ct, :],
                        in_=wbf[wk][:, ct * 128:(ct + 1) * 128],
                        transpose=True,
                    )

            # ---- Phase 2a: q/k projections, feature-major qT/kT [e, tok] ----
            qT = big.tile([128, ET, NT], BF16)
            kT = big.tile([128, ET, NT], BF16)
            for et in range(ET):
                for n in range(NT // NCHUNK):
                    ns = slice(n * NCHUNK, (n + 1) * NCHUNK)
                    for dst, wk, brow in ((qT, "q", 0), (kT, "k", 1)):
                        ps = ps_qk.tile([128, NCHUNK], F32, tag="ps_qk")
                        for ct in range(CT):
                            nc.tensor.matmul(
                                ps[:, :],
                                lhsT=WT[wk][:, ct, et * 128:(et + 1) * 128],
                                rhs=XT[:, ct, ns],
                                start=(ct == 0),
                                stop=(ct == CT - 1),
                            )
                        nc.vector.tensor_scalar_add(
                            dst[:, et, ns], ps[:, :], bqk[:, brow, et:et + 1]
                        )

            # ---- Phase 2b: v projection, token-major with ones column ----
            # v_sb[tok%128, tok//128, h, 0:64] = v ; [..., 64] = 1.0
            v_sb = big.tile([128, NT // 128, H, HD + 1], BF16)
            nc.vector.memset(v_sb[:, :, :, HD:HD + 1], 1.0)
            for pt in range(NT // 128):
                for e0, ew in ((0, 512), (512, 256)):
                    ps = ps_v.tile([128, 512], F32, tag="ps_v")
                    for ct in range(CT):
                        nc.tensor.matmul(
                            ps[:, :ew],
                            lhsT=XT[:, ct, pt * 128:(pt + 1) * 128],
                            rhs=WT["v"][:, ct, e0:e0 + ew],
                            start=(ct == 0),
                            stop=False,
                        )
                    nc.tensor.matmul(
                        ps[:, :ew],
                        lhsT=ones[:, :],
                        rhs=bvr[:, e0:e0 + ew],
                        start=False,
                        stop=True,
                    )
                    nc.vector.tensor_copy(
                        out=v_sb[:, pt, e0 // HD:(e0 + ew) // HD, 0:HD],
                        in_=ps[:, :ew].rearrange("p (h d) -> p h d", d=HD),
                    )

            # ---- Phase 3: attention per (seq, head) ----
            for s in range(NSEQ):
                os_tiles = [out_pool.tile([128, D], F32, tag="os", name=f"os_{s}_{i}") for i in range(2)]
                for h in range(H):
                    et_h = h // 2
                    off = (h % 2) * HD
                    tok0 = s * P_
                    # dotsT[q, p] = k_h @ q_h^T  (one K=64 matmul per q-tile)
                    psd = ps_d.tile([128, 2, 256], F32, tag="ps_d")
                    for qt in range(2):
                        nc.tensor.matmul(
                            psd[:, qt, :],
                            lhsT=kT[off:off + HD, et_h, tok0 + qt * 128:tok0 + (qt + 1) * 128],
                            rhs=qT[off:off + HD, et_h, tok0:tok0 + P_],
                            start=True,
                            stop=True,
                        )
                    # exp (scaled) -> attnT bf16
                    at = attn_pool.tile([128, 2, 256], BF16, tag="at")
                    for qt in range(2):
                        nc.scalar.activation(
                            out=at[:, qt, :],
                            in_=psd[:, qt, :],
                            func=mybir.ActivationFunctionType.Exp,
                            scale=SCALE,
                        )
                    # out[p, 0:64] = attnT.T @ v_aug ; col 64 accumulates the
                    # softmax denominator via v's ones column
                    psa = ps_av.tile([128, 2, HD + 1], F32, tag="ps_av")
                    for pt2 in range(2):
                        for qt in range(2):
                            nc.tensor.matmul(
                                psa[:, pt2, :],
                                lhsT=at[:, qt, pt2 * 128:(pt2 + 1) * 128],
                                rhs=v_sb[:, s * 2 + qt, h, :],
                                start=(qt == 0),
                                stop=(qt == 1),
                            )
                        rec = rec_pool.tile([128, 1], F32, tag="rec")
                        nc.vector.reciprocal(rec[:, :], psa[:, pt2, HD:HD + 1])
                        nc.vector.tensor_scalar_mul(
                            os_tiles[pt2][:, h * HD:(h + 1) * HD],
                            psa[:, pt2, 0:HD],
                            rec[:, :],
                        )
                for pt2 in range(2):
                    r0 = (s * 2 + pt2) * 128
                    nc.sync.dma_start(out=out[r0:r0 + 128, :], in_=os_tiles[pt2][:, :])

    nc.finalize()
    return nc


_NC_CACHE = {}


def _get_nc():
    if "nc" not in _NC_CACHE:
        _NC_CACHE["nc"] = build_nc()
    return _NC_CACHE["nc"]


def kernel(x, Wq, bq, Wk, bk, Wv, bv):
    x = np.ascontiguousarray(np.asarray(x, dtype=np.float32))
    args = {
        "Wq": np.ascontiguousarray(np.asarray(Wq, dtype=np.float32)),
        "Wk": np.ascontiguousarray(np.asarray(Wk, dtype=np.float32)),
        "Wv": np.ascontiguousarray(np.asarray(Wv, dtype=np.float32)),
        "bq": np.ascontiguousarray(np.asarray(bq, dtype=np.float32)),
        "bk": np.ascontiguousarray(np.asarray(bk, dtype=np.float32)),
        "bv": np.ascontiguousarray(np.asarray(bv, dtype=np.float32)),
    }
    xf = x.reshape(B * T_ * L * P_, D)
    nc = _get_nc()
    in_maps = [
        {"x": xf[i * NT:(i + 1) * NT], **args} for i in range(N_CORES)
    ]
    res = run_bass_kernel_spmd(nc, in_maps, list(range(N_CORES)))
    outs = [res.results[i]["out"] for i in range(N_CORES)]
    full = np.concatenate(outs, axis=0).reshape(B, T_, L, P_, D)
    return full.astype(np.float32)


# revision 4
# speedup vs baseline: 1.3912x; 1.3912x over previous
"""Multi-head attention (QKV projection + softmax attention) on 8 TRN2 NeuronCores.

Reference computation (per full input):
    x: [2, 8, 4, 256, 768] fp32, H=12 heads, head_dim=64
    q = split_heads(x @ Wq.T + bq); k, v likewise
    out = softmax(q k^T / sqrt(64)) v, heads merged back -> [2, 8, 4, 256, 768]

Sharding: data-parallel over the 2*8*4 = 64 independent (b,t,l) sequences,
8 sequences per core; weights replicated.

Per-core kernel design (all matmuls bf16 inputs, fp32 PSUM accumulate):
  - x and W are cast fp32->bf16 during SWDGE DMA into DRAM scratch, then
    loaded transposed via the HWDGE xbar DMA-transpose, giving XT [c, tok]
    and WT [c, e] (contraction-major) in SBUF.
  - qT, kT computed feature-major ([e, tok]) so the q k^T matmul reads them
    directly; v computed token-major ([tok, e]) so attn @ v reads it directly.
  - v bias is added via a K=1 matmul row (ones lhsT, bv rhs) which is exact
    because softmax rows sum to one; q/k biases via per-partition
    tensor_scalar adds.
  - softmax skips the max-subtraction (logits are ~N(0,1); exp is safe) and
    gets its denominator for free from a ones-column appended to v, so
    normalization is one reciprocal + one broadcast multiply.
  - work is software-pipelined in 512-token chunks (projection of chunk c+1
    is emitted before attention of chunk c) so ScalarE exp and VectorE
    epilogues overlap TensorE projections and the PE stays HAM-warm.
  - dots matmuls are K=64; heads are processed in (even, odd) pairs whose
    operands live at partition offsets 0/64, so the two matmuls run
    concurrently in disjoint PE row groups.
"""

import sys

for _p in ("/opt/trn_rl_repo",):
    if _p not in sys.path:
        sys.path.insert(0, _p)

import numpy as np

import concourse.bass as bass
import concourse.tile as tile
from concourse import bacc, mybir
from concourse.bass_utils import run_bass_kernel_spmd

N_CORES = 8
B, T_, L, P_, D = 2, 8, 4, 256, 768
H = 12
HD = D // H          # 64
NSEQ = (B * T_ * L) // N_CORES   # 8 sequences per core
NT = NSEQ * P_       # 2048 tokens per core
CT = D // 128        # 6 contraction tiles
ET = D // 128        # 6 output-feature tiles
NCHUNK = 512         # tokens per pipeline chunk
NPIPE = NT // NCHUNK # 4 chunks
SCALE = float(HD) ** -0.5
HG = 6               # heads per PSUM attn-output group

F32 = mybir.dt.float32
BF16 = mybir.dt.bfloat16


def build_nc():
    nc = bacc.Bacc()

    x = nc.dram_tensor("x", [NT, D], F32, kind="ExternalInput")
    Wq = nc.dram_tensor("Wq", [D, D], F32, kind="ExternalInput")
    Wk = nc.dram_tensor("Wk", [D, D], F32, kind="ExternalInput")
    Wv = nc.dram_tensor("Wv", [D, D], F32, kind="ExternalInput")
    bq = nc.dram_tensor("bq", [D], F32, kind="ExternalInput")
    bk = nc.dram_tensor("bk", [D], F32, kind="ExternalInput")
    bv = nc.dram_tensor("bv", [D], F32, kind="ExternalInput")
    out = nc.dram_tensor("out", [NT, D], F32, kind="ExternalOutput")

    # bf16 DRAM scratch for the xbar transpose loads
    xbf = nc.dram_tensor("xbf", [NT, D], BF16)
    wbf = {
        "q": nc.dram_tensor("wqbf", [D, D], BF16),
        "k": nc.dram_tensor("wkbf", [D, D], BF16),
        "v": nc.dram_tensor("wvbf", [D, D], BF16),
    }
    w_in = {"q": Wq, "k": Wk, "v": Wv}

    with tile.TileContext(nc) as tc:
        with (
            tc.tile_pool(name="const", bufs=1) as const,
            tc.tile_pool(name="big", bufs=1) as big,
            tc.tile_pool(name="attn", bufs=6) as attn_pool,
            tc.tile_pool(name="rec", bufs=8) as rec_pool,
            tc.tile_pool(name="outp", bufs=4) as out_pool,
            tc.tile_pool(name="ps_qk", bufs=2, space="PSUM") as ps_qk,
            tc.tile_pool(name="ps_v", bufs=2, space="PSUM") as ps_v,
            tc.tile_pool(name="ps_d", bufs=2, space="PSUM") as ps_d,
            tc.tile_pool(name="ps_av", bufs=2, space="PSUM") as ps_av,
        ):
            # ---- constants ----
            bqk = const.tile([128, 2, ET], F32)
            nc.sync.dma_start(out=bqk[:, 0, :], in_=bq.rearrange("(t p) -> p t", p=128))
            nc.sync.dma_start(out=bqk[:, 1, :], in_=bk.rearrange("(t p) -> p t", p=128))
            bvr = const.tile([1, D], BF16)
            nc.gpsimd.dma_start(out=bvr[:, :], in_=bv.rearrange("(o d) -> o d", o=1))
            ones = const.tile([1, 128], BF16)
            nc.vector.memset(ones[:, :], 1.0)

            # ---- weight casts (SWDGE); q and x-chunk0 first: they gate start ----
            nc.gpsimd.dma_start(out=wbf["q"][:, :], in_=w_in["q"][:, :])
            nc.gpsimd.dma_start(out=xbf[0:NCHUNK, :], in_=x[0:NCHUNK, :])
            nc.gpsimd.dma_start(out=wbf["k"][:, :], in_=w_in["k"][:, :])
            nc.gpsimd.dma_start(out=wbf["v"][:, :], in_=w_in["v"][:, :])

            # ---- transposed weight loads; q on the SP HWDGE queue, k/v on ACT ----
            WT = {}
            for wk, eng in (("q", nc.sync), ("k", nc.scalar), ("v", nc.scalar)):
                WT[wk] = big.tile([128, CT, D], BF16, name=f"WT_{wk}")
                for ct in range(CT):
                    eng.dma_start(
                        out=WT[wk][:, ct, :],
                        in_=wbf[wk][:, ct * 128:(ct + 1) * 128],
                        transpose=True,
                    )

            XT = big.tile([128, CT, NT], BF16)     # XT[c%128, c//128, tok]
            qT = big.tile([128, ET, NT], BF16)
            kT = big.tile([128, ET, NT], BF16)
            # v_sb[tok%128, tok//128, h, 0:64] = v ; [..., 64] = 1.0
            v_sb = big.tile([128, NT // 128, H, HD + 1], BF16)
            nc.vector.memset(v_sb[:, :, :, HD:HD + 1], 1.0)

            def emit_cast(c):
                r = slice(c * NCHUNK, (c + 1) * NCHUNK)
                nc.gpsimd.dma_start(out=xbf[r, :], in_=x[r, :])

            def emit_transpose(c):
                for ct in range(CT):
                    nc.sync.dma_start(
                        out=XT[:, ct, c * NCHUNK:(c + 1) * NCHUNK],
                        in_=xbf[c * NCHUNK:(c + 1) * NCHUNK, ct * 128:(ct + 1) * 128],
                        transpose=True,
                    )

            def emit_proj(c):
                ns = slice(c * NCHUNK, (c + 1) * NCHUNK)
                # q/k feature-major
                for dst, wk, brow in ((qT, "q", 0), (kT, "k", 1)):
                    for et in range(ET):
                        ps = ps_qk.tile([128, NCHUNK], F32, tag="ps_qk", name="ps_qk")
                        for ct in range(CT):
                            nc.tensor.matmul(
                                ps[:, :],
                                lhsT=WT[wk][:, ct, et * 128:(et + 1) * 128],
                                rhs=XT[:, ct, ns],
                                start=(ct == 0),
                                stop=(ct == CT - 1),
                            )
                        nc.vector.tensor_scalar_add(
                            dst[:, et, ns], ps[:, :], bqk[:, brow, et:et + 1]
                        )
                # v token-major with folded bias
                for pt in range(c * 4, (c + 1) * 4):
                    for e0, ew in ((0, 512), (512, 256)):
                        ps = ps_v.tile([128, 512], F32, tag="ps_v", name="ps_v")
                        for ct in range(CT):
                            nc.tensor.matmul(
                                ps[:, :ew],
                                lhsT=XT[:, ct, pt * 128:(pt + 1) * 128],
                                rhs=WT["v"][:, ct, e0:e0 + ew],
                                start=(ct == 0),
                                stop=False,
                            )
                        nc.tensor.matmul(
                            ps[:, :ew],
                            lhsT=ones[:, :],
                            rhs=bvr[:, e0:e0 + ew],
                            start=False,
                            stop=True,
                        )
                        nc.vector.tensor_copy(
                            out=v_sb[:, pt, e0 // HD:(e0 + ew) // HD, 0:HD],
                            in_=ps[:, :ew].rearrange("p (h d) -> p h d", d=HD),
                        )

            def emit_attn_seq(s):
                tok0 = s * P_
                os_tiles = [
                    out_pool.tile([128, D], F32, tag="os", name=f"os_{s}_{i}")
                    for i in range(2)
                ]
                for g in range(H // HG):     # head groups sharing a PSUM bank
                    pav = [
                        ps_av.tile([128, HG, HD + 1], F32, tag="ps_av",
                                   name=f"pav_{s}_{g}_{i}")
                        for i in range(2)
                    ]
                    ats = []
                    for jp in range(HG // 2):   # (even, odd) head pairs
                        h0 = g * HG + jp * 2
                        et_h = h0 // 2
                        psd = [
                            ps_d.tile([128, 2, 256], F32, tag="ps_d",
                                      name=f"psd_{s}_{h0}_{i}")
                            for i in range(2)
                        ]
                        # dotsT[q, p] = k_h @ q_h^T ; the pair's matmuls sit at
                        # partition offsets 0/64 -> disjoint PE row groups
                        for qt in range(2):
                            for i in range(2):
                                off = i * HD
                                nc.tensor.matmul(
                                    psd[i][:, qt, :],
                                    lhsT=kT[off:off + HD, et_h,
                                            tok0 + qt * 128:tok0 + (qt + 1) * 128],
                                    rhs=qT[off:off + HD, et_h, tok0:tok0 + P_],
                                    start=True,
                                    stop=True,
                                )
                        for i in range(2):
                            at = attn_pool.tile([128, 2, 256], BF16, tag="at",
                                                name=f"at_{s}_{h0}_{i}")
                            nc.scalar.activation(
                                out=at[:, :, :],
                                in_=psd[i][:, :, :],
                                func=mybir.ActivationFunctionType.Exp,
                                scale=SCALE,
                            )
                            ats.append(at)
                    # attn @ v_aug for the 6 heads of this group
                    for j in range(HG):
                        h = g * HG + j
                        at = ats[j]
                        for pt2 in range(2):
                            for qt in range(2):
                                nc.tensor.matmul(
                                    pav[pt2][:, j, :],
                                    lhsT=at[:, qt, pt2 * 128:(pt2 + 1) * 128],
                                    rhs=v_sb[:, s * 2 + qt, h, :],
                                    start=(qt == 0),
                                    stop=(qt == 1),
                                )
                    for pt2 in range(2):
                        rec = rec_pool.tile([128, HG, 1], F32, tag="rec",
                                            name=f"rec_{s}_{g}_{pt2}")
                        nc.vector.reciprocal(rec[:, :, :], pav[pt2][:, :, HD:HD + 1])
                        rec_b = bass.AP(
                            tensor=rec.tensor,
                            offset=rec.offset,
                            ap=[rec.ap[0], rec.ap[1], [0, HD]],
                        )
                        nc.vector.tensor_mul(
                            os_tiles[pt2][:, g * HG * HD:(g + 1) * HG * HD]
                            .rearrange("p (h d) -> p h d", d=HD),
                            pav[pt2][:, :, 0:HD],
                            rec_b,
                        )
                for pt2 in range(2):
                    r0 = (s * 2 + pt2) * 128
                    nc.sync.dma_start(out=out[r0:r0 + 128, :], in_=os_tiles[pt2][:, :])

            # ---- software pipeline over chunks ----
            emit_transpose(0)
            emit_proj(0)
            for c in range(NPIPE):
                if c + 1 < NPIPE:
                    emit_cast(c + 1)
                    emit_transpose(c + 1)
                    emit_proj(c + 1)
                for sloc in range(NCHUNK // P_):
                    emit_attn_seq(c * (NCHUNK // P_) + sloc)

    nc.finalize()
    return nc


_NC_CACHE = {}


def _get_nc():
    if "nc" not in _NC_CACHE:
        _NC_CACHE["nc"] = build_nc()
    return _NC_CACHE["nc"]


def kernel(x, Wq, bq, Wk, bk, Wv, bv):
    x = np.ascontiguousarray(np.asarray(x, dtype=np.float32))
    args = {
        "Wq": np.ascontiguousarray(np.asarray(Wq, dtype=np.float32)),
        "Wk": np.ascontiguousarray(np.asarray(Wk, dtype=np.float32)),
        "Wv": np.ascontiguousarray(np.asarray(Wv, dtype=np.float32)),
        "bq": np.ascontiguousarray(np.asarray(bq, dtype=np.float32)),
        "bk": np.ascontiguousarray(np.asarray(bk, dtype=np.float32)),
        "bv": np.ascontiguousarray(np.asarray(bv, dtype=np.float32)),
    }
    xf = x.reshape(B * T_ * L * P_, D)
    nc = _get_nc()
    in_maps = [
        {"x": xf[i * NT:(i + 1) * NT], **args} for i in range(N_CORES)
    ]
    res = run_bass_kernel_spmd(nc, in_maps, list(range(N_CORES)))
    outs = [res.results[i]["out"] for i in range(N_CORES)]
    full = np.concatenate(outs, axis=0).reshape(B, T_, L, P_, D)
    return full.astype(np.float32)
